# revision 3
# baseline (speedup 1.0000x reference)
"""Trainium2 Bass kernel for the neural 2D min-sum (normalized offset-free)
LDPC decoder nn.Module problem.

Strategy
--------
Data-parallel over the batch: B=512 codewords split as 64 per NeuronCore
(8 cores).  Per core, all per-edge state lives in SBUF with the graph on
the partition axis and the 64-batch on the free axis (256B rows).

The Tanner graph (from edge_v / edge_c) is 6-regular on checks and
3-regular on variables.  Edges are stored "slot-separated": 6 arrays of
shape [128, 32, 64] (check c at partition c%128, block c//128), so the
check-node min-sum update is fully element-wise across the 6 slots
(leave-one-out min via pair/prefix mins, sign product via multiplies).

The only irregular data movement is the per-iteration check->variable
crossing.  c2v is DMAd contiguously to DRAM, then 12 dma_gather
instructions (4096 rows x 256B each, indices precomputed on host) fetch,
for every edge, the c2v of the two *other* edges of its variable:
    v2c_new = llr_e + alpha_t * (ga + gb)
(the self term cancels algebraically).  alpha/beta are baked as
immediates (the program is compiled after inputs are known).

The host wrapper derives all index tables from edge_v/edge_c with
numpy (argsort), shards/transposes llr, runs the SPMD kernel on 8
cores, and inverse-permutes/transposes the outputs.
"""

import sys

for _p in ("/opt/trn_rl_repo",):
    if _p not in sys.path:
        sys.path.insert(0, _p)

import numpy as np

import concourse.bass as bass
import concourse.bacc as bacc
import concourse.mybir as mybir
import concourse.tile as tile
from concourse.bass_utils import run_bass_kernel_spmd

# Problem constants (hardcoded per the harness contract)
N = 8192          # variable nodes
M = 4096          # check nodes
DC = 6            # check degree (slots)
DV = 3            # variable degree
E = N * DV        # 24576 edges
B = 512           # batch
T = 10            # BP iterations
NCORES = 8
BL = B // NCORES  # 64 batch rows per core
PB = 128          # partitions
GB_ = M // PB     # 32 blocks per slot array
CHUNK_BLKS = 4    # check-phase chunk width in blocks (256 cols)
HALF_BLKS = GB_ // 2

F32 = mybir.dt.float32
I32 = mybir.dt.int32
I16 = mybir.dt.int16
ALU = mybir.AluOpType
ACTF = mybir.ActivationFunctionType


def _derive_graph(edge_v: np.ndarray, edge_c: np.ndarray):
    """Host-side index derivation. Requires dc=6-regular checks and
    dv=3-regular variables (guaranteed by the reference construction)."""
    edge_v = np.asarray(edge_v, dtype=np.int64)
    edge_c = np.asarray(edge_c, dtype=np.int64)
    assert edge_v.shape == (E,) and edge_c.shape == (E,)

    order = np.argsort(edge_c, kind="stable")
    assert (edge_c[order] == np.repeat(np.arange(M), DC)).all(), (
        "graph is not 6-regular on checks"
    )
    slot_edge = order.reshape(M, DC).T.copy()  # [DC, M]: edge id of (slot j, check c)

    vorder = np.argsort(edge_v, kind="stable")
    assert (edge_v[vorder] == np.repeat(np.arange(N), DV)).all(), (
        "graph is not 3-regular on variables"
    )
    var_edges = vorder.reshape(N, DV)  # ascending edge ids per variable

    # position (slot j, check c) of each edge
    j_of_e = np.empty(E, dtype=np.int64)
    c_of_e = np.empty(E, dtype=np.int64)
    for j in range(DC):
        j_of_e[slot_edge[j]] = j
        c_of_e[slot_edge[j]] = np.arange(M)
    # DRAM/SBUF storage row of each edge in the c2v buffer:
    #   slot-major, then p-major within the slot (p = c%128, g = c//128)
    srow = j_of_e * M + (c_of_e % PB) * GB_ + (c_of_e // PB)

    # the two *other* edges of each edge's variable (ascending)
    oth = np.empty((E, 2), dtype=np.int64)
    oth[var_edges[:, 0]] = var_edges[:, [1, 2]]
    oth[var_edges[:, 1]] = var_edges[:, [0, 2]]
    oth[var_edges[:, 2]] = var_edges[:, [0, 1]]

    idx_a = np.empty((DC, M), dtype=np.int16)
    idx_b = np.empty((DC, M), dtype=np.int16)
    idx_l = np.empty((DC, M), dtype=np.int16)
    for j in range(DC):
        e = slot_edge[j]
        idx_a[j] = srow[oth[e, 0]]
        idx_b[j] = srow[oth[e, 1]]
        idx_l[j] = edge_v[e]

    # output mapping: first edge of each variable -> (j, p, g)
    e0 = var_edges[:, 0]
    out_j = j_of_e[e0]
    out_p = c_of_e[e0] % PB
    out_g = c_of_e[e0] // PB
    return idx_l, idx_a, idx_b, out_j, out_p, out_g


def _wrap_idx(idx_m: np.ndarray) -> np.ndarray:
    """dma_gather index layout: list position k lives at [k%16, k//16],
    replicated across the 8 groups of 16 partitions."""
    w = idx_m.reshape(M // 16, 16).T  # [16, M/16]
    return np.tile(w, (PB // 16, 1)).copy()


def _build_program(alpha: np.ndarray, beta: np.ndarray) -> bacc.Bacc:
    nc = bacc.Bacc()

    llr_t = nc.dram_tensor("llr_t", [N, BL], F32, kind="ExternalInput").ap()
    ixl_d = nc.dram_tensor("ixl", [DC, PB, M // 16], I16, kind="ExternalInput").ap()
    ixa_d = nc.dram_tensor("ixa", [DC, PB, M // 16], I16, kind="ExternalInput").ap()
    ixb_d = nc.dram_tensor("ixb", [DC, PB, M // 16], I16, kind="ExternalInput").ap()
    post_d = nc.dram_tensor("post", [DC, PB, GB_, BL], F32, kind="ExternalOutput").ap()
    bits_d = nc.dram_tensor("bits", [DC, PB, GB_, BL], I32, kind="ExternalOutput").ap()
    # c2v staging in DRAM (ping-pong across iterations), row = j*M + p*32 + g
    cd = [
        nc.dram_tensor("cda", [E, BL], F32).ap(),
        nc.dram_tensor("cdb", [E, BL], F32).ap(),
    ]
    cdv = [c.rearrange("(j p g) e -> j p g e", j=DC, p=PB) for c in cd]

    with tile.TileContext(nc) as tc:
        with (
            tc.tile_pool(name="persist", bufs=1) as pp,
            tc.tile_pool(name="cs", bufs=1) as csp,
            tc.tile_pool(name="gb", bufs=3) as gbp,
            tc.tile_pool(name="bits", bufs=1) as bip,
            tc.tile_pool(name="tmp", bufs=1) as tp,
        ):
            # --- persistent tiles ---
            ixl = [pp.tile([PB, M // 16], I16, tag=f"ixl{j}", name=f"ixl{j}") for j in range(DC)]
            ixa = [pp.tile([PB, M // 16], I16, tag=f"ixa{j}", name=f"ixa{j}") for j in range(DC)]
            ixb = [pp.tile([PB, M // 16], I16, tag=f"ixb{j}", name=f"ixb{j}") for j in range(DC)]
            for j in range(DC):
                nc.sync.dma_start(ixl[j][:], ixl_d[j])
                nc.sync.dma_start(ixa[j][:], ixa_d[j])
                nc.sync.dma_start(ixb[j][:], ixb_d[j])

            LP = [pp.tile([PB, GB_, BL], F32, tag=f"lp{j}", name=f"lp{j}") for j in range(DC)]
            for j in range(DC):
                nc.gpsimd.dma_gather(
                    LP[j][:], llr_t, ixl[j][:], M, M, BL, single_packet=False
                )
            V = [pp.tile([PB, GB_, BL], F32, tag=f"v{j}", name=f"v{j}") for j in range(DC)]

            def check_chunk(xs, blk0, nblk, cs_list, cs_off, beta_t):
                """min-sum check update for column blocks [blk0, blk0+nblk);
                reads 6 slot arrays xs, writes c2v into cs_list at cs_off."""
                w = nblk * BL
                xsl = [x[:, blk0 : blk0 + nblk, :] for x in xs]
                mg = [tp.tile([PB, w], F32, tag=f"m{j}", name=f"m{j}") for j in range(DC)]
                sg = [tp.tile([PB, w], F32, tag=f"s{j}", name=f"s{j}") for j in range(DC)]
                pq = [tp.tile([PB, w], F32, tag=f"pq{i}", name=f"pq{i}") for i in range(6)]
                ex = [tp.tile([PB, w], F32, tag=f"e{j}", name=f"e{j}") for j in range(DC)]
                for j in range(DC):
                    nc.scalar.activation(mg[j][:], xsl[j], ACTF.Abs)
                    nc.scalar.activation(sg[j][:], xsl[j], ACTF.Sign)
                # pair mins and leave-one-pair-out mins
                nc.vector.tensor_tensor(pq[0][:], mg[0][:], mg[1][:], ALU.min)  # P01
                nc.vector.tensor_tensor(pq[1][:], mg[2][:], mg[3][:], ALU.min)  # P23
                nc.vector.tensor_tensor(pq[2][:], mg[4][:], mg[5][:], ALU.min)  # P45
                nc.vector.tensor_tensor(pq[3][:], pq[1][:], pq[2][:], ALU.min)  # Q0
                nc.vector.tensor_tensor(pq[4][:], pq[0][:], pq[2][:], ALU.min)  # Q1
                nc.vector.tensor_tensor(pq[5][:], pq[0][:], pq[1][:], ALU.min)  # Q2
                # leave-one-out mins
                nc.vector.tensor_tensor(ex[0][:], mg[1][:], pq[3][:], ALU.min)
                nc.vector.tensor_tensor(ex[1][:], mg[0][:], pq[3][:], ALU.min)
                nc.vector.tensor_tensor(ex[2][:], mg[3][:], pq[4][:], ALU.min)
                nc.vector.tensor_tensor(ex[3][:], mg[2][:], pq[4][:], ALU.min)
                nc.vector.tensor_tensor(ex[4][:], mg[5][:], pq[5][:], ALU.min)
                nc.vector.tensor_tensor(ex[5][:], mg[4][:], pq[5][:], ALU.min)
                # sign product * beta (pq tiles are dead now; reuse)
                nc.vector.tensor_tensor(pq[0][:], sg[0][:], sg[1][:], ALU.mult)
                nc.vector.tensor_tensor(pq[1][:], sg[2][:], sg[3][:], ALU.mult)
                nc.vector.tensor_tensor(pq[2][:], sg[4][:], sg[5][:], ALU.mult)
                nc.vector.tensor_tensor(pq[3][:], pq[0][:], pq[1][:], ALU.mult)
                nc.vector.tensor_tensor(pq[4][:], pq[3][:], pq[2][:], ALU.mult)
                nc.vector.tensor_scalar(pq[4][:], pq[4][:], float(beta_t), None, ALU.mult)
                # c2v = (sign_j * beta*sprod) * exclmin_j
                for j in range(DC):
                    csl = cs_list[j][:, cs_off : cs_off + nblk, :].rearrange(
                        "p b e -> p (b e)"
                    )
                    nc.vector.tensor_tensor(csl, sg[j][:], pq[4][:], ALU.mult)
                    nc.vector.tensor_tensor(csl, csl, ex[j][:], ALU.mult)

            for t in range(T):
                beta_t = float(beta[t])
                alpha_t = float(alpha[t])
                cdt = cd[t % 2]
                cdvt = cdv[t % 2]
                xs = [LP[j][:] if t == 0 else V[j][:] for j in range(DC)]

                # --- check phase: two halves, chunked ---
                for half in range(2):
                    cs_list = [
                        csp.tile([PB, HALF_BLKS, BL], F32, tag=f"cs{j}", name=f"cs{j}")
                        for j in range(DC)
                    ]
                    h0 = half * HALF_BLKS
                    for ck in range(HALF_BLKS // CHUNK_BLKS):
                        check_chunk(
                            xs,
                            h0 + ck * CHUNK_BLKS,
                            CHUNK_BLKS,
                            cs_list,
                            ck * CHUNK_BLKS,
                            beta_t,
                        )
                    for j in range(DC):
                        nc.sync.dma_start(
                            cdvt[j][:, h0 : h0 + HALF_BLKS, :], cs_list[j][:]
                        )

                if t < T - 1:
                    # --- variable phase: v2c = llr_e + alpha*(ga+gb) ---
                    for j in range(DC):
                        nc.gpsimd.dma_gather(
                            V[j][:], cdt, ixa[j][:], M, M, BL, single_packet=False
                        )
                        gb = gbp.tile([PB, GB_, BL], F32, tag="gb", name="gb")
                        nc.gpsimd.dma_gather(
                            gb[:], cdt, ixb[j][:], M, M, BL, single_packet=False
                        )
                        nc.vector.tensor_tensor(V[j][:], V[j][:], gb[:], ALU.add)
                        nc.vector.scalar_tensor_tensor(
                            V[j][:], V[j][:], alpha_t, LP[j][:], ALU.mult, ALU.add
                        )
                else:
                    # --- final: posterior = llr + (ga + gb + c2v_self) ---
                    for j in range(DC):
                        nc.gpsimd.dma_gather(
                            V[j][:], cdt, ixa[j][:], M, M, BL, single_packet=False
                        )
                        gb = gbp.tile([PB, GB_, BL], F32, tag="gb", name="gb")
                        nc.gpsimd.dma_gather(
                            gb[:], cdt, ixb[j][:], M, M, BL, single_packet=False
                        )
                        cself = gbp.tile([PB, GB_, BL], F32, tag="gb", name="gb")
                        nc.sync.dma_start(cself[:], cdvt[j])
                        nc.vector.tensor_tensor(V[j][:], V[j][:], gb[:], ALU.add)
                        nc.vector.tensor_tensor(V[j][:], V[j][:], cself[:], ALU.add)
                        nc.vector.tensor_tensor(V[j][:], V[j][:], LP[j][:], ALU.add)
                        bt = bip.tile([PB, GB_, BL], I32, tag="bt", name="bt")
                        nc.vector.tensor_scalar(
                            bt[:], V[j][:], 0.0, None, ALU.is_lt
                        )
                        nc.sync.dma_start(post_d[j], V[j][:])
                        nc.sync.dma_start(bits_d[j], bt[:])

    nc.compile()
    return nc


def _prepare(llr, edge_v, edge_c, beta, alpha):
    idx_l, idx_a, idx_b, out_j, out_p, out_g = _derive_graph(edge_v, edge_c)
    ixl = np.stack([_wrap_idx(idx_l[j]) for j in range(DC)])
    ixa = np.stack([_wrap_idx(idx_a[j]) for j in range(DC)])
    ixb = np.stack([_wrap_idx(idx_b[j]) for j in range(DC)])

    llr = np.asarray(llr, dtype=np.float32)
    in_maps = []
    for k in range(NCORES):
        llr_t = np.ascontiguousarray(llr[k * BL : (k + 1) * BL].T)  # [N, BL]
        in_maps.append({"llr_t": llr_t, "ixl": ixl, "ixa": ixa, "ixb": ixb})
    return in_maps, (out_j, out_p, out_g)


def _assemble(results, out_map):
    out_j, out_p, out_g = out_map
    posterior = np.empty((B, N), dtype=np.float32)
    bits = np.empty((B, N), dtype=np.int32)
    for k in range(NCORES):
        pd = results[k]["post"]  # [DC, PB, GB_, BL]
        bd = results[k]["bits"]
        posterior[k * BL : (k + 1) * BL] = pd[out_j, out_p, out_g, :].T
        bits[k * BL : (k + 1) * BL] = bd[out_j, out_p, out_g, :].T
    return bits, posterior


def _run(llr, edge_v, edge_c, beta, alpha, trace=False, tmpdir=None):
    in_maps, out_map = _prepare(llr, edge_v, edge_c, beta, alpha)
    nc = _build_program(np.asarray(alpha, np.float32), np.asarray(beta, np.float32))
    res = run_bass_kernel_spmd(
        nc, in_maps, list(range(NCORES)), trace=trace, tmpdir=tmpdir
    )
    return _assemble(res.results, out_map), res


def kernel(llr, edge_v, edge_c, beta, alpha):
    (bits, posterior), _ = _run(llr, edge_v, edge_c, beta, alpha, trace=False)
    return bits, posterior


# revision 5
# speedup vs baseline: 3.0550x; 3.0550x over previous
"""Trainium2 Bass kernel for the neural 2D min-sum LDPC decoder problem.

Strategy (v2)
-------------
Data-parallel over the batch: B=512 codewords, 64 per NeuronCore (8 cores).
Per core, per-edge state lives in SBUF with the graph on the partition axis
and the 64-batch on the free axis (256B rows).

The Tanner graph (edge_v/edge_c) is 6-regular on checks, 3-regular on
variables, built from 3 "layers": sorting each check's edges by edge id
puts exactly one edge of every variable in slots {0,1}, {2,3}, {4,5}.
Variables are relabeled by their slot-{0,1} position, which makes the
layer-0 part of both crossings contiguous.

Per iteration:
  check phase   x_j = u_j - alpha_{t-1}*c2v_j (fused), then leave-one-out
                min + sign-product min-sum -> c2v (6 slot arrays
                [128,32,64]); slots 2..5 DMA'd contiguously to DRAM.
  crossing 1    4 dma_gathers (4096x256B) fetch, for every variable, the
                c2v of its layer-1 and layer-2 edges;
                u_var = llr + alpha_t*((c2v_l0 + g_mid) + g_hi).
  crossing 2    u_var written contiguously to DRAM; 4 dma_gathers
                redistribute it to slots 2..5 position order (layer 0 is
                contiguous by construction).
All gathers: 256B rows, 4 SWDGE queues round-robin, single_packet=False,
split in halves so the next phase starts on the first half early.
alpha/beta are baked as immediates (compiled after inputs are known).
"""

import sys

for _p in ("/opt/trn_rl_repo",):
    if _p not in sys.path:
        sys.path.insert(0, _p)

import numpy as np

import concourse.bass as bass
import concourse.bacc as bacc
import concourse.mybir as mybir
import concourse.tile as tile
from concourse.bass_utils import run_bass_kernel_spmd

N = 8192          # variable nodes
M = 4096          # check nodes
DC = 6            # check degree (slots)
DV = 3            # variable degree
E = N * DV
B = 512
T = 10
NCORES = 8
BL = B // NCORES  # 64
PB = 128
GB_ = M // PB     # 32 blocks per slot array
CHUNK_BLKS = 4
NCHUNK = GB_ // CHUNK_BLKS

F32 = mybir.dt.float32
I32 = mybir.dt.int32
I16 = mybir.dt.int16
ALU = mybir.AluOpType
ACTF = mybir.ActivationFunctionType


def _derive_graph(edge_v: np.ndarray, edge_c: np.ndarray):
    """Host-side index derivation (layered 6-regular/3-regular graph)."""
    edge_v = np.asarray(edge_v, dtype=np.int64)
    edge_c = np.asarray(edge_c, dtype=np.int64)
    assert edge_v.shape == (E,) and edge_c.shape == (E,)

    order = np.argsort(edge_c, kind="stable")
    assert (edge_c[order] == np.repeat(np.arange(M), DC)).all(), (
        "graph is not 6-regular on checks"
    )
    slot_edge = order.reshape(M, DC).T.copy()  # [DC, M] edge id at (slot j, check c)

    # per-edge position
    j_of_e = np.empty(E, dtype=np.int64)
    c_of_e = np.empty(E, dtype=np.int64)
    for j in range(DC):
        j_of_e[slot_edge[j]] = j
        c_of_e[slot_edge[j]] = np.arange(M)

    # each variable must have exactly one edge in slots {0,1}, {2,3}, {4,5}
    layer_of_e = j_of_e // 2
    ve = np.full((N, 3), -1, dtype=np.int64)
    for lay in range(3):
        sel = np.where(layer_of_e == lay)[0]
        vs = edge_v[sel]
        assert len(np.unique(vs)) == N, f"layer {lay} is not a permutation"
        ve[vs, lay] = sel
    assert (ve >= 0).all()

    # storage row helpers (p-major: row = (c%128)*32 + c//128)
    rowmaj = (c_of_e % PB) * GB_ + (c_of_e // PB)
    # c2v DRAM buffer holds slots 2..5 only
    cdrow = (j_of_e - 2) * M + rowmaj          # valid for slots 2..5
    # u/llr DRAM row of a variable = its slot-{0,1} position
    fr_of_e = j_of_e * M + rowmaj              # valid for slots 0..1
    fr_of_v = fr_of_e[ve[:, 0]]                # [N]

    # u-build gathers (dst = parity pi, list pos = check c): variable at
    # (j=pi, c) -> cdram rows of its layer-1 / layer-2 edges
    ix1 = np.empty((2, M), dtype=np.int16)
    ix2 = np.empty((2, M), dtype=np.int16)
    # crossing-2 gathers (dst slot j=2..5, list pos = c): udram row of v(j,c)
    ixu = np.empty((4, M), dtype=np.int16)
    for pi in range(2):
        e = slot_edge[pi]                      # layer-0 edge at (pi, c)
        v = edge_v[e]
        ix1[pi] = cdrow[ve[v, 1]]
        ix2[pi] = cdrow[ve[v, 2]]
    for j in range(2, DC):
        v = edge_v[slot_edge[j]]
        ixu[j - 2] = fr_of_v[v]

    # host llr/output mapping: variable id at each u/llr DRAM row
    vid_of_fr = np.empty(N, dtype=np.int64)
    vid_of_fr[fr_of_v] = np.arange(N)
    return ix1, ix2, ixu, vid_of_fr


def _wrap_idx(idx_m: np.ndarray) -> np.ndarray:
    """dma_gather index layout: list position k at [k%16, k//16],
    replicated across the 8 groups of 16 partitions."""
    w = idx_m.reshape(M // 16, 16).T
    return np.tile(w, (PB // 16, 1)).copy()


def _build_program(alpha: np.ndarray, beta: np.ndarray) -> bacc.Bacc:
    nc = bacc.Bacc(num_swdge_queues=4)

    llr_t = nc.dram_tensor("llr_t", [N, BL], F32, kind="ExternalInput").ap()
    ix1_d = nc.dram_tensor("ix1", [2, PB, M // 16], I16, kind="ExternalInput").ap()
    ix2_d = nc.dram_tensor("ix2", [2, PB, M // 16], I16, kind="ExternalInput").ap()
    ixu_d = nc.dram_tensor("ixu", [4, PB, M // 16], I16, kind="ExternalInput").ap()
    post_d = nc.dram_tensor("post", [2, PB, GB_, BL], F32, kind="ExternalOutput").ap()
    bits_d = nc.dram_tensor("bits", [2, PB, GB_, BL], I32, kind="ExternalOutput").ap()
    # c2v slots 2..5, ping-pong; u_var, ping-pong
    cdrs = [
        nc.dram_tensor("cda", [4 * M, BL], F32).ap(),
        nc.dram_tensor("cdb", [4 * M, BL], F32).ap(),
    ]
    udrs = [
        nc.dram_tensor("uda", [N, BL], F32).ap(),
        nc.dram_tensor("udb", [N, BL], F32).ap(),
    ]
    cdrv = [c.rearrange("(j p g) e -> j p g e", j=4, p=PB) for c in cdrs]
    udrv = [u.rearrange("(pi p g) e -> pi p g e", pi=2, p=PB) for u in udrs]

    QN = [0]

    def qn():
        QN[0] = (QN[0] + 1) % 4
        return QN[0]

    with tile.TileContext(nc) as tc:
        with (
            tc.tile_pool(name="persist", bufs=1) as pp,
            tc.tile_pool(name="gbp", bufs=3) as gbp,
            tc.tile_pool(name="bits", bufs=1) as bip,
            tc.tile_pool(name="tmp", bufs=1) as tp,
        ):
            ix1 = [pp.tile([PB, M // 16], I16, tag=f"ix1{i}", name=f"ix1{i}") for i in range(2)]
            ix2 = [pp.tile([PB, M // 16], I16, tag=f"ix2{i}", name=f"ix2{i}") for i in range(2)]
            ixu = [pp.tile([PB, M // 16], I16, tag=f"ixu{i}", name=f"ixu{i}") for i in range(4)]
            for i in range(2):
                nc.sync.dma_start(ix1[i][:], ix1_d[i])
                nc.sync.dma_start(ix2[i][:], ix2_d[i])
            for i in range(4):
                nc.sync.dma_start(ixu[i][:], ixu_d[i])

            # llr in variable(-row) order, split by parity
            LV = [pp.tile([PB, GB_, BL], F32, tag=f"lv{i}", name=f"lv{i}") for i in range(2)]
            for i in range(2):
                nc.sync.dma_start(
                    LV[i][:], llr_t.rearrange("(pi p g) e -> pi p g e", pi=2, p=PB)[i]
                )
            # u in position order: UP[0/1] = u_var parities, UP[2..5] gathered
            UP = [pp.tile([PB, GB_, BL], F32, tag=f"up{j}", name=f"up{j}") for j in range(DC)]
            # c2v slot arrays
            C = [pp.tile([PB, GB_, BL], F32, tag=f"c{j}", name=f"c{j}") for j in range(DC)]

            # t=0 init: u = llr (position order); slots 2..5 gathered from llr_t
            for h in range(2):
                for i in range(4):
                    nc.gpsimd.dma_gather(
                        UP[2 + i][:, h * 16 : (h + 1) * 16, :],
                        llr_t,
                        ixu[i][:, h * 128 : (h + 1) * 128],
                        M // 2,
                        M // 2,
                        BL,
                        single_packet=False,
                        queue_num=qn(),
                    )

            def check_chunk(t, ck, beta_t, alpha_p):
                """min-sum check update for chunk ck (CHUNK_BLKS blocks)."""
                w = CHUNK_BLKS * BL
                b0 = ck * CHUNK_BLKS
                xt = [tp.tile([PB, w], F32, tag=f"x{j}", name=f"x{j}") for j in range(DC)]
                xs = []
                for j in range(DC):
                    usl = (LV[j][:, b0 : b0 + CHUNK_BLKS, :] if t == 0 and j < 2
                           else UP[j][:, b0 : b0 + CHUNK_BLKS, :])
                    if t == 0:
                        xs.append(usl)
                    else:
                        csl = C[j][:, b0 : b0 + CHUNK_BLKS, :]
                        nc.vector.scalar_tensor_tensor(
                            xt[j][:], csl, -alpha_p, usl, ALU.mult, ALU.add
                        )
                        xs.append(xt[j][:])
                mg = [tp.tile([PB, w], F32, tag=f"m{j}", name=f"m{j}") for j in range(DC)]
                sg = [tp.tile([PB, w], F32, tag=f"s{j}", name=f"s{j}") for j in range(DC)]
                pq = [tp.tile([PB, w], F32, tag=f"pq{i}", name=f"pq{i}") for i in range(6)]
                for j in range(DC):
                    nc.scalar.activation(mg[j][:], xs[j], ACTF.Abs)
                    nc.scalar.activation(sg[j][:], xs[j], ACTF.Sign)
                nc.vector.tensor_tensor(pq[0][:], mg[0][:], mg[1][:], ALU.min)
                nc.vector.tensor_tensor(pq[1][:], mg[2][:], mg[3][:], ALU.min)
                nc.vector.tensor_tensor(pq[2][:], mg[4][:], mg[5][:], ALU.min)
                nc.vector.tensor_tensor(pq[3][:], pq[1][:], pq[2][:], ALU.min)  # Q0
                nc.vector.tensor_tensor(pq[4][:], pq[0][:], pq[2][:], ALU.min)  # Q1
                nc.vector.tensor_tensor(pq[5][:], pq[0][:], pq[1][:], ALU.min)  # Q2
                ex = xt  # x tiles are dead; reuse their slots for excl mins
                nc.vector.tensor_tensor(ex[0][:], mg[1][:], pq[3][:], ALU.min)
                nc.vector.tensor_tensor(ex[1][:], mg[0][:], pq[3][:], ALU.min)
                nc.vector.tensor_tensor(ex[2][:], mg[3][:], pq[4][:], ALU.min)
                nc.vector.tensor_tensor(ex[3][:], mg[2][:], pq[4][:], ALU.min)
                nc.vector.tensor_tensor(ex[4][:], mg[5][:], pq[5][:], ALU.min)
                nc.vector.tensor_tensor(ex[5][:], mg[4][:], pq[5][:], ALU.min)
                # sign product * beta
                nc.vector.tensor_tensor(pq[0][:], sg[0][:], sg[1][:], ALU.mult)
                nc.vector.tensor_tensor(pq[1][:], sg[2][:], sg[3][:], ALU.mult)
                nc.vector.tensor_tensor(pq[2][:], sg[4][:], sg[5][:], ALU.mult)
                nc.vector.tensor_tensor(pq[3][:], pq[0][:], pq[1][:], ALU.mult)
                nc.vector.tensor_tensor(pq[4][:], pq[3][:], pq[2][:], ALU.mult)
                nc.vector.tensor_scalar(pq[4][:], pq[4][:], float(beta_t), None, ALU.mult)
                # c2v_j = (sign_j * beta*sprod) * exclmin_j
                for j in range(DC):
                    csl = C[j][:, b0 : b0 + CHUNK_BLKS, :].rearrange("p b e -> p (b e)")
                    nc.vector.tensor_tensor(csl, sg[j][:], pq[4][:], ALU.mult)
                    nc.vector.tensor_tensor(csl, csl, ex[j][:], ALU.mult)

            for t in range(T):
                beta_t = float(beta[t])
                alpha_t = float(alpha[t])
                alpha_p = float(alpha[t - 1]) if t > 0 else 0.0
                cdt, cdvt = cdrs[t % 2], cdrv[t % 2]
                udt, udvt = udrs[t % 2], udrv[t % 2]

                # --- check phase (8 chunks), c2v slots 2..5 -> DRAM by halves
                for ck in range(NCHUNK):
                    check_chunk(t, ck, beta_t, alpha_p)
                    if ck == NCHUNK // 2 - 1:
                        for j in range(2, DC):
                            nc.sync.dma_start(
                                cdvt[j - 2][:, :16, :], C[j][:, :16, :]
                            )
                for j in range(2, DC):
                    nc.sync.dma_start(cdvt[j - 2][:, 16:, :], C[j][:, 16:, :])

                if t < T - 1:
                    # --- crossing 1 + u build (by halves) ---
                    for h in range(2):
                        hs = slice(h * 16, (h + 1) * 16)
                        ls = slice(h * 128, (h + 1) * 128)
                        gm = [None, None]
                        gh = [None, None]
                        for pi in range(2):
                            gm[pi] = gbp.tile([PB, 16, BL], F32, tag="gm", name="gm")
                            gh[pi] = gbp.tile([PB, 16, BL], F32, tag="gh", name="gh")
                            nc.gpsimd.dma_gather(
                                gm[pi][:], cdt, ix1[pi][:, ls], M // 2, M // 2, BL,
                                single_packet=False, queue_num=qn(),
                            )
                            nc.gpsimd.dma_gather(
                                gh[pi][:], cdt, ix2[pi][:, ls], M // 2, M // 2, BL,
                                single_packet=False, queue_num=qn(),
                            )
                        for pi in range(2):
                            up = UP[pi][:, hs, :].rearrange("p b e -> p (b e)")
                            cp = C[pi][:, hs, :].rearrange("p b e -> p (b e)")
                            nc.vector.tensor_tensor(up, cp, gm[pi][:].rearrange("p b e -> p (b e)"), ALU.add)
                            nc.vector.tensor_tensor(up, up, gh[pi][:].rearrange("p b e -> p (b e)"), ALU.add)
                            nc.vector.scalar_tensor_tensor(
                                up, up, alpha_t,
                                LV[pi][:, hs, :].rearrange("p b e -> p (b e)"),
                                ALU.mult, ALU.add,
                            )
                            nc.sync.dma_start(udvt[pi][:, hs, :], UP[pi][:, hs, :])
                    # --- crossing 2: u -> position order, slots 2..5 ---
                    for h in range(2):
                        for i in range(4):
                            nc.gpsimd.dma_gather(
                                UP[2 + i][:, h * 16 : (h + 1) * 16, :],
                                udt,
                                ixu[i][:, h * 128 : (h + 1) * 128],
                                M // 2, M // 2, BL,
                                single_packet=False, queue_num=qn(),
                            )
                else:
                    # --- final: posterior = llr + (c2v_l0 + g_mid + g_hi) ---
                    for h in range(2):
                        hs = slice(h * 16, (h + 1) * 16)
                        ls = slice(h * 128, (h + 1) * 128)
                        gm = [None, None]
                        gh = [None, None]
                        for pi in range(2):
                            gm[pi] = gbp.tile([PB, 16, BL], F32, tag="gm", name="gm")
                            gh[pi] = gbp.tile([PB, 16, BL], F32, tag="gh", name="gh")
                            nc.gpsimd.dma_gather(
                                gm[pi][:], cdt, ix1[pi][:, ls], M // 2, M // 2, BL,
                                single_packet=False, queue_num=qn(),
                            )
                            nc.gpsimd.dma_gather(
                                gh[pi][:], cdt, ix2[pi][:, ls], M // 2, M // 2, BL,
                                single_packet=False, queue_num=qn(),
                            )
                        for pi in range(2):
                            up = UP[pi][:, hs, :].rearrange("p b e -> p (b e)")
                            cp = C[pi][:, hs, :].rearrange("p b e -> p (b e)")
                            nc.vector.tensor_tensor(up, cp, gm[pi][:].rearrange("p b e -> p (b e)"), ALU.add)
                            nc.vector.tensor_tensor(up, up, gh[pi][:].rearrange("p b e -> p (b e)"), ALU.add)
                            nc.vector.tensor_tensor(
                                up, up, LV[pi][:, hs, :].rearrange("p b e -> p (b e)"), ALU.add
                            )
                            bt = bip.tile([PB, 16, BL], I32, tag="bt", name="bt")
                            nc.vector.tensor_scalar(
                                bt[:], UP[pi][:, hs, :], 0.0, None, ALU.is_lt
                            )
                            nc.sync.dma_start(post_d[pi][:, hs, :], UP[pi][:, hs, :])
                            nc.sync.dma_start(bits_d[pi][:, hs, :], bt[:])

    nc.compile()
    return nc


def _prepare(llr, edge_v, edge_c, beta, alpha):
    ix1, ix2, ixu, vid_of_fr = _derive_graph(edge_v, edge_c)
    ix1w = np.stack([_wrap_idx(ix1[i]) for i in range(2)])
    ix2w = np.stack([_wrap_idx(ix2[i]) for i in range(2)])
    ixuw = np.stack([_wrap_idx(ixu[i]) for i in range(4)])

    llr = np.asarray(llr, dtype=np.float32)
    in_maps = []
    for k in range(NCORES):
        llr_t = np.ascontiguousarray(llr[k * BL : (k + 1) * BL, vid_of_fr].T)
        in_maps.append({"llr_t": llr_t, "ix1": ix1w, "ix2": ix2w, "ixu": ixuw})
    return in_maps, vid_of_fr


def _assemble(results, vid_of_fr):
    posterior = np.empty((B, N), dtype=np.float32)
    bits = np.empty((B, N), dtype=np.int32)
    for k in range(NCORES):
        pd = results[k]["post"].reshape(N, BL)  # row = pi*4096 + p*32 + g
        bd = results[k]["bits"].reshape(N, BL)
        posterior[k * BL : (k + 1) * BL, vid_of_fr] = pd.T
        bits[k * BL : (k + 1) * BL, vid_of_fr] = bd.T
    return bits, posterior


def _run(llr, edge_v, edge_c, beta, alpha, trace=False, tmpdir=None):
    in_maps, vid_of_fr = _prepare(llr, edge_v, edge_c, beta, alpha)
    nc = _build_program(np.asarray(alpha, np.float32), np.asarray(beta, np.float32))
    res = run_bass_kernel_spmd(
        nc, in_maps, list(range(NCORES)), trace=trace, tmpdir=tmpdir
    )
    return _assemble(res.results, vid_of_fr), res


def kernel(llr, edge_v, edge_c, beta, alpha):
    (bits, posterior), _ = _run(llr, edge_v, edge_c, beta, alpha, trace=False)
    return bits, posterior
